# revision 19
# baseline (speedup 1.0000x reference)
"""Trainium2 Bass kernel for nn_Cell_46042049413406 (quantized 2-layer conv1d).

Sharding: pure data-parallel over batch: 16 batches -> 8 cores x 2 batches.

Wall-clock structure (axon-tunneled cores, link ~30-40MB/s, ~100ms/transfer
fixed): the dominant cost is host<->device traffic, so
  - x is fake-quantized to int8 ON HOST (exact same integers the reference's
    fake_quant produces scaled by 128) and shipped as int8: 134MB -> 33.5MB.
  - z is produced as int8 on device and dequantized (k/128) on host:
    67MB -> 16.8MB d2h.
  - the donated zero output buffers are created ON DEVICE (jnp.zeros under
    jit with sharded out_shardings) instead of shipping 67MB of host zeros.
  - the PJRT execute wrapper (jit of shard_map'ed bass_exec) is built ONCE
    and cached; bass2jax.run_bass_via_pjrt rebuilds it every call.
  - folded/quantized weights and their device-resident globals are cached
    keyed by a hash of the small param tensors.
  - quantize + device_put of x shards runs in 8 threads (overlaps the
    per-transfer fixed cost); fetch + dequant of z shards likewise.

Per-core layout: xq [2,4,R] int8 is viewed as 128 SBUF partitions (b,i,c16)
each holding a contiguous chunk of S = L/16 positions.  Both convs run on the
TensorEngine as 3 shift-matmuls (taps t=0,1,2) with block-diagonal fp16
weight matrices; halo columns come straight from the (host-padded)
contiguous DRAM reads, so no transposes are needed anywhere.

All arithmetic is exact-integer-in-float: quantized activations/weights are
small integers, fp16 products are exact, fp32 PSUM accumulation is exact.
fake_quant floors are computed exactly via:
  x-path: on host (numpy f32 ops match XLA f32 ops bit-for-bit)
  y-path: +2^-8 nudge, then fp16-write cast rounds RNE at ulp=1 in [1024,2048)
  z-path: +2^-8 nudge, then +3*2^22 magic add rounds RNE at ulp=1, -(MAGIC+128)
          and an exact f32->int8 write cast
"""
import sys

sys.path.insert(0, "/opt/trn_rl_repo")

import hashlib
from concurrent.futures import ThreadPoolExecutor

import numpy as np

B, CIN, L = 16, 4, 524288
S = L // 16          # 32768 chunk length
F = 256              # sweep tile width
NT = S // F          # 128 tiles
R = L + 4            # host-padded row length (2 zeros each side)
NCORES = 8
MAGIC = float(3 * 2**22)          # 12582912.0
NUDGE = 2.0**-8
INV128 = np.float32(0.0078125)

_CACHED = {}
_POOL = ThreadPoolExecutor(NCORES)


def _fake_quant_np(x, bits=8):
    s = np.float32(2.0 ** (bits - 1))
    return np.clip(np.floor(x * s + np.float32(0.5)), -s, s - 1).astype(np.float32) / s


def _fold_weights(w1, b1, gamma, beta, bn_mean, bn_var, w2, b2):
    """Reproduce the reference's folded/quantized params (fp32, on CPU jax to
    match XLA rsqrt bit-for-bit; falls back to numpy if jax unavailable)."""
    try:
        import jax
        import jax.numpy as jnp
        from jax import lax

        cpu = jax.devices("cpu")[0]

        def fq(x, bits):
            s = jnp.asarray(2.0 ** (bits - 1), x.dtype)
            return jnp.clip(jnp.floor(x * s + 0.5), -s, s - 1.0) / s

        with jax.default_device(cpu):
            sf = jnp.asarray(gamma) * lax.rsqrt(jnp.asarray(bn_var) + 1e-5)
            wq = fq(jnp.asarray(w1) * sf[:, None, None], 8)
            bq = fq((jnp.asarray(b1) - jnp.asarray(bn_mean)) * sf + jnp.asarray(beta), 8)
            w2q = fq(jnp.asarray(w2), 8)
            b2q = fq(jnp.asarray(b2), 8)
            return (np.asarray(wq), np.asarray(bq), np.asarray(w2q), np.asarray(b2q))
    except Exception:
        sf = gamma / np.sqrt(bn_var + np.float32(1e-5))
        return (
            _fake_quant_np(w1 * sf[:, None, None]),
            _fake_quant_np((b1 - bn_mean) * sf + beta),
            _fake_quant_np(w2),
            _fake_quant_np(b2),
        )


def build_nc(Lk=L):
    """Build the SPMD Bass program for one core (2 batches, length Lk).

    Input xq int8 [2,4,Lk+4] (already fake-quantized*128, zero-padded 2 each
    side); output z int8 [2,2,Lk] (z*128)."""
    import concourse.bass as bass
    import concourse.bacc as bacc
    import concourse.mybir as mybir
    from concourse.bass_types import AP
    from concourse.tile import TileContext

    Sk = Lk // 16
    NTk = Sk // F
    Rk = Lk + 4
    f32, f16, i8 = mybir.dt.float32, mybir.dt.float16, mybir.dt.int8

    nc = bacc.Bacc("TRN2", target_bir_lowering=False, debug=False)
    xp = nc.dram_tensor("xp", (2, CIN, Rk), i8, kind="ExternalInput").ap()
    w1l = nc.dram_tensor("w1l", (128, 3 * 128), f16, kind="ExternalInput").ap()
    w2l = nc.dram_tensor("w2l", (128, 3 * 32), f16, kind="ExternalInput").ap()
    bvec = nc.dram_tensor("bvec", (128, 3), f32, kind="ExternalInput").ap()
    z = nc.dram_tensor("z", (2, 2, Lk), i8, kind="ExternalOutput").ap()

    AOP = mybir.AluOpType
    AF = mybir.ActivationFunctionType

    with TileContext(nc) as tc:
        with (
            tc.tile_pool(name="const", bufs=1) as cpool,
            tc.tile_pool(name="work", bufs=4) as wp,
            tc.tile_pool(name="ypool", bufs=4) as yp,
            tc.tile_pool(name="zpool", bufs=3) as zp,
            tc.tile_pool(name="psy", bufs=2, space="PSUM") as psy,
            tc.tile_pool(name="psz", bufs=2, space="PSUM") as psz,
        ):
            w1t = cpool.tile([128, 3 * 128], f16, tag="w1t")
            nc.sync.dma_start(w1t[:], w1l[:])
            w2t = cpool.tile([128, 3 * 32], f16, tag="w2t")
            nc.sync.dma_start(w2t[:], w2l[:])
            bt = cpool.tile([128, 3], f32, tag="bt")
            nc.sync.dma_start(bt[:], bvec[:])
            tc.strict_bb_all_engine_barrier()

            psum_z = None
            n0_even = 0
            for jj in range(NTk // 2):
                n0p = jj * 2 * F
                # ---- load x double-tile [128, 2F+4] int8, cast to f16
                xt = wp.tile([128, 2 * F + 4], i8, tag="xt")
                src = AP(tensor=xp.tensor, offset=n0p,
                         ap=[[CIN * Rk, 2], [Rk, CIN], [Sk, 16], [1, 2 * F + 4]])
                nc.gpsimd.dma_start(xt[:], src)
                xq = wp.tile([128, 2 * F + 4], f16, tag="xq")
                nc.gpsimd.tensor_copy(xq[:], xt[:])
                for h in (0, 1):
                    j = jj * 2 + h
                    n0 = j * F
                    # ---- conv1: per batch, 3 shift matmuls, K=64 -> M=128
                    psum_y = [psy.tile([128, F + 2], f32, name=f"py{b}_{j}", tag=f"y{b}") for b in (0, 1)]
                    for s in range(3):
                        for b in (0, 1):
                            nc.tensor.matmul(
                                psum_y[b][:],
                                w1t[b * 64:(b + 1) * 64, s * 128:(s + 1) * 128],
                                xq[b * 64:(b + 1) * 64, h * F + s:h * F + s + F + 2],
                                start=(s == 0), stop=(s == 2),
                                tile_position=(b * 64, 0),
                            )
                    # ---- y fake-quant -> rhs2 fp16 (value = yq + 1152)
                    rhs2 = []
                    for b in (0, 1):
                        u = yp.tile([128, F + 2], f32, name=f"u{b}_{j}", tag=f"u{b}")
                        nc.scalar.activation(u[:], psum_y[b][:], AF.Relu,
                                             bias=bt[:, 1:2], scale=0.0078125)
                        r2 = yp.tile([128, F + 2], f16, name=f"r{b}_{j}", tag=f"r{b}")
                        nc.vector.tensor_scalar(r2[:], u[:], 255.25, 1024.0,
                                                AOP.min, AOP.add)
                        rhs2.append(r2)

                    # ---- conv2: col-tiled into psum_z quadrant cg = b*2+par
                    par = j & 1
                    if par == 0:
                        psum_z = psz.tile([128, F], f32, name=f"pz_{j}", tag="z")
                        n0_even = n0
                    for s in range(3):
                        for b in (0, 1):
                            cg = b * 2 + par
                            nc.tensor.matmul(
                                psum_z[cg * 32:(cg + 1) * 32, :],
                                w2t[:, s * 32:(s + 1) * 32],
                                rhs2[b][:, s:s + F],
                                start=(s == 0), stop=(s == 2),
                                tile_position=(0, cg * 32),
                                skip_group_check=True,
                            )
                    if par == 1:
                        # ---- z fake-quant + int8 store
                        zv = zp.tile([128, F], f32, name=f"zv_{j}", tag="zv")
                        nc.scalar.activation(zv[:], psum_z[:], AF.Relu,
                                             bias=bt[:, 2:3], scale=0.0078125)
                        zt = zp.tile([128, F], f32, name=f"zt_{j}", tag="zt")
                        nc.vector.tensor_scalar(zt[:], zv[:], 255.25, MAGIC,
                                                AOP.min, AOP.add)
                        zq = zp.tile([128, F], i8, name=f"zq_{j}", tag="zq")
                        nc.vector.tensor_scalar(zq[:], zt[:], -(MAGIC + 128.0),
                                                None, AOP.add)
                        for b in (0, 1):
                            dst = AP(tensor=z.tensor, offset=b * 2 * Lk + n0_even,
                                     ap=[[F, 2], [Lk, 2], [Sk, 16], [1, F]])
                            nc.sync.dma_start(dst, zq[b * 64:(b + 1) * 64, :])
    nc.compile()
    return nc


def _host_prep(w1, b1, gamma, beta, bn_mean, bn_var, w2, b2):
    wq, bq, w2q, b2q = _fold_weights(w1, b1, gamma, beta, bn_mean, bn_var, w2, b2)
    m1 = np.round(wq * 128.0).astype(np.int32)      # [8,4,3]
    m2 = np.round(w2q * 128.0).astype(np.int32)     # [2,8,3]
    mb1 = np.round(bq * 128.0).astype(np.int32)     # [8]
    mb2 = np.round(b2q * 128.0).astype(np.int32)    # [2]

    a1 = np.zeros((128, 3 * 128), np.float16)
    for s in range(3):
        for i in range(CIN):
            for o in range(8):
                for c in range(16):
                    v = np.float16(float(m1[o, i, s]))
                    a1[i * 16 + c, s * 128 + o * 16 + c] = v
                    a1[64 + i * 16 + c, s * 128 + o * 16 + c] = v
    a2 = np.zeros((128, 3 * 32), np.float16)
    for s in range(3):
        for o in range(8):
            for c2 in range(2):
                for c in range(16):
                    a2[o * 16 + c, s * 32 + c2 * 16 + c] = np.float16(float(m2[c2, o, s]))

    bvec = np.zeros((128, 3), np.float32)
    bvec[:, 0] = 0.5
    for o in range(8):
        for c in range(16):
            bvec[o * 16 + c, 1] = np.float32(float(mb1[o]) + 128.0 + NUDGE)
    m2sum = m2.sum(axis=(1, 2))                     # [2]
    for b in range(2):
        for par in range(2):
            for c2 in range(2):
                for c in range(16):
                    p = b * 64 + par * 32 + c2 * 16 + c
                    bvec[p, 2] = np.float32(
                        -9.0 * float(m2sum[c2]) + float(mb2[c2]) + 128.0 + NUDGE)
    return (wq, bq, w2q, b2q), a1, a2, bvec


def _edge_fix(out, x, wq, bq, w2q, b2q):
    """Reference zero-pads y between convs; the kernel extrapolates conv1 into
    the halo instead.  Only output positions 0 and Lk-1 differ - recompute
    them on host with exact fp32 integer arithmetic."""
    fq = _fake_quant_np
    Lk = x.shape[2]
    for side in (0, 1):
        xs = x[:, :, :3] if side == 0 else x[:, :, Lk - 3:]
        xqs = fq(xs)                                  # [B,4,3]
        xpad = np.zeros((x.shape[0], CIN, 5), np.float32)
        xpad[:, :, 1:4] = xqs
        # y at the two positions adjacent to the edge
        ys = np.zeros((x.shape[0], 8, 2), np.float32)  # pos (0,1) or (L-2,L-1)
        for k in range(2):
            base = k if side == 0 else k + 1
            acc = np.zeros((x.shape[0], 8), np.float32)
            for o in range(8):
                for i in range(CIN):
                    for t in range(3):
                        acc[:, o] += wq[o, i, t] * xpad[:, i, base + t]
            ys[:, :, k] = fq(acc + bq[None, :])
        ypad = np.zeros((x.shape[0], 8, 4), np.float32)
        ypad[:, :, 1:3] = ys
        zpos = 0 if side == 0 else Lk - 1
        ybase = 0 if side == 0 else 1
        acc = np.zeros((x.shape[0], 2), np.float32)
        for c2 in range(2):
            for o in range(8):
                for t in range(3):
                    acc[:, c2] += w2q[c2, o, t] * ypad[:, o, ybase + t]
        out[:, :, zpos] = fq(acc + b2q[None, :])


PUT_CONC = int(__import__("os").environ.get("KPUT", "3"))


def _runner_state(nc, n_cores):
    """Build (once) the cached per-core PJRT execute wrapper: input/output
    specs, a single jitted body reused on all devices (jax caches one
    executable per device placement), and per-device on-device zeros makers
    for the donated output buffers."""
    st = _CACHED.get("runner")
    if st is not None:
        return st

    import jax
    import jax.numpy as jnp
    from jax.sharding import SingleDeviceSharding

    from concourse import bass2jax
    import concourse.mybir as mybir

    bass2jax.install_neuronx_cc_hook()

    assert nc.dbg_addr is None or not nc.dbg_callbacks
    partition_name = nc.partition_id_tensor.name if nc.partition_id_tensor else None
    dbg_name = nc.dbg_addr.name if nc.dbg_addr is not None else None

    in_names, out_names, out_avals = [], [], []
    for alloc in nc.m.functions[0].allocations:
        if not isinstance(alloc, mybir.MemoryLocationSet):
            continue
        name = alloc.memorylocations[0].name
        if alloc.kind == "ExternalInput":
            if name != partition_name:
                in_names.append(name)
        elif alloc.kind == "ExternalOutput":
            shape = tuple(alloc.tensor_shape)
            dtype = mybir.dt.np(alloc.dtype)
            out_names.append(name)
            out_avals.append(jax.core.ShapedArray(shape, dtype))
    n_params = len(in_names)
    n_outs = len(out_avals)

    bind_names = list(in_names) + list(out_names)
    if partition_name is not None:
        bind_names.append(partition_name)

    devices = jax.devices()[:n_cores]
    assert len(devices) == n_cores

    def _body(*args):
        operands = list(args)
        if partition_name is not None:
            operands.append(bass2jax.partition_id_tensor())
        outs = bass2jax._bass_exec_p.bind(
            *operands,
            out_avals=tuple(out_avals),
            in_names=tuple(bind_names),
            out_names=tuple(out_names),
            lowering_input_output_aliases=(),
            sim_require_finite=True,
            sim_require_nnan=True,
            nc=nc,
        )
        return tuple(outs)

    donate = tuple(range(n_params, n_params + n_outs))
    body_jit = jax.jit(_body, donate_argnums=donate, keep_unused=True)

    zshapes = [tuple(a.shape) for a in out_avals]
    zdtypes = [a.dtype for a in out_avals]

    def _mk_zeros():
        return tuple(jnp.zeros(s, d) for s, d in zip(zshapes, zdtypes))

    zeros_fns = [
        jax.jit(_mk_zeros,
                out_shardings=(SingleDeviceSharding(devices[c]),) * n_outs)
        for c in range(n_cores)
    ]

    # ---- single-dispatch variant: shard_map over the 8-core mesh
    from jax.experimental.shard_map import shard_map
    from jax.sharding import Mesh, NamedSharding, PartitionSpec

    mesh = Mesh(np.asarray(devices), ("core",))
    ns = NamedSharding(mesh, PartitionSpec("core"))
    in_specs = (PartitionSpec("core"),) * (n_params + n_outs)
    out_specs = (PartitionSpec("core"),) * n_outs
    sharded = jax.jit(
        shard_map(_body, mesh=mesh, in_specs=in_specs, out_specs=out_specs,
                  check_rep=False),
        donate_argnums=donate,
        keep_unused=True,
    )

    def _mk_gzeros():
        return tuple(jnp.zeros((n_cores * s[0], *s[1:]), d)
                     for s, d in zip(zshapes, zdtypes))

    gzeros_fn = jax.jit(_mk_gzeros, out_shardings=(ns,) * n_outs)

    st = dict(
        jax=jax, n_cores=n_cores, devices=devices,
        in_names=in_names, out_names=out_names, out_avals=out_avals,
        body_jit=body_jit, zeros_fns=zeros_fns, dbg_name=dbg_name,
        mesh=mesh, ns=ns, sharded=sharded, gzeros_fn=gzeros_fn,
    )
    _CACHED["runner"] = st
    return st


def _fast_run_via_pjrt(nc, in_maps, n_cores):
    """Drop-in replacement for bass2jax.run_bass_via_pjrt (axon path) with a
    per-core pipeline: each core's inputs upload (bounded concurrency so the
    cores are staggered), its NEFF dispatches as soon as its shard lands, and
    its outputs download while later cores are still uploading.  Values in
    in_maps may be jax.Arrays already resident on the right device (cached
    replicated weights); numpy values are uploaded here."""
    import jax
    import threading

    st = _runner_state(nc, n_cores)
    devices = st["devices"]
    body_jit = st["body_jit"]
    sem = threading.Semaphore(PUT_CONC)

    import time as _time
    trace = [] if __import__("os").environ.get("KTIME") else None
    t00 = _time.perf_counter()

    import os

    def mark(c, tag, t0):
        if trace is not None:
            trace.append((c, tag, t0 - t00, _time.perf_counter() - t00))

    def upload_core(c):
        vals = []
        for name in st["in_names"]:
            if name == st["dbg_name"] and name not in in_maps[c]:
                v = np.zeros((1, 2), np.uint32)
            else:
                v = in_maps[c][name]
            if isinstance(v, jax.Array):
                vals.append(v)
            else:
                stash = _CACHED.setdefault("xstash", {})
                if os.environ.get("KFAKEPUT") and (c, name) in stash:
                    vals.append(stash[(c, name)])
                    continue
                with sem:
                    t0 = _time.perf_counter()
                    if callable(v):
                        v = v()          # lazy host prep (e.g. quantize)
                    mark(c, "quant", t0)
                    t0 = _time.perf_counter()
                    a = jax.device_put(np.asarray(v), devices[c])
                    a.block_until_ready()
                    mark(c, "put", t0)
                stash[(c, name)] = a
                vals.append(a)
        return vals

    def finish(c, outs_np):
        pp = _CACHED.get("core_post")
        if pp is not None:
            outs_np = pp(c, outs_np)
        return outs_np

    if os.environ.get("KMODE", "global") == "percore":
        def core_task(c):
            vals = upload_core(c)
            zeros = st["zeros_fns"][c]()
            t0 = _time.perf_counter()
            outs = body_jit(*vals, *zeros)
            if os.environ.get("KFAKEFETCH"):
                for o in outs:
                    o.block_until_ready()
                return [None for _ in outs]
            outs_np = [np.asarray(o) for o in outs]
            mark(c, "exec+fetch", t0)
            return finish(c, outs_np)

        futs = [_POOL.submit(core_task, c) for c in range(n_cores)]
        per_core_outs = [f.result() for f in futs]
    else:
        # single dispatch: upload all shards (threads), assemble global
        # arrays, one shard_map exec, then per-shard threaded fetch
        futs = [_POOL.submit(upload_core, c) for c in range(n_cores)]
        per_vals = [f.result() for f in futs]
        t0 = _time.perf_counter()
        global_ins = []
        for i in range(len(st["in_names"])):
            per = [per_vals[c][i] for c in range(n_cores)]
            sh = per[0].shape
            global_ins.append(jax.make_array_from_single_device_arrays(
                (n_cores * sh[0], *sh[1:]), st["ns"], per))
        zeros = st["gzeros_fn"]()
        out_arrs = st["sharded"](*global_ins, *zeros)
        mark(0, "dispatch", t0)

        shard_map_per_out = []
        for i in range(len(st["out_names"])):
            shards = sorted(out_arrs[i].addressable_shards,
                            key=lambda s: (s.index[0].start or 0))
            assert len(shards) == n_cores
            shard_map_per_out.append(shards)

        def fetch_task(c):
            t0 = _time.perf_counter()
            if os.environ.get("KFAKEFETCH"):
                for i in range(len(st["out_names"])):
                    shard_map_per_out[i][c].data.block_until_ready()
                return [None for _ in st["out_names"]]
            outs_np = [np.asarray(shard_map_per_out[i][c].data)
                       for i in range(len(st["out_names"]))]
            mark(c, "fetch", t0)
            return finish(c, outs_np)

        futs = [_POOL.submit(fetch_task, c) for c in range(n_cores)]
        per_core_outs = [f.result() for f in futs]

    results = []
    for c in range(n_cores):
        outs = per_core_outs[c]
        results.append({name: outs[i] for i, name in enumerate(st["out_names"])})
    if trace is not None:
        for c, tag, a, b in sorted(trace, key=lambda r: r[2]):
            print(f"  core{c} {tag:10s} {a*1e3:7.1f} -> {b*1e3:7.1f} ms "
                  f"({(b-a)*1e3:6.1f})", flush=True)
    return results


def _install_patch():
    if _CACHED.get("patched"):
        return
    from concourse import bass2jax

    bass2jax.run_bass_via_pjrt = _fast_run_via_pjrt
    _CACHED["patched"] = True


def _param_key(*arrs):
    h = hashlib.sha1()
    for a in arrs:
        h.update(np.ascontiguousarray(a).tobytes())
    return h.hexdigest()


def kernel(x, w1, b1, gamma, beta, bn_mean, bn_var, w2, b2):
    import jax

    from concourse import bass_utils

    _install_patch()

    x = np.asarray(x)
    if x.dtype != np.float32:
        x = x.astype(np.float32)

    params = [np.asarray(a, np.float32) for a in
              (w1, b1, gamma, beta, bn_mean, bn_var, w2, b2)]
    pk = _param_key(*params)
    if _CACHED.get("pk") != pk:
        folded, a1, a2, bvec = _host_prep(*params)
        _CACHED["pk"] = pk
        _CACHED["folded"] = folded
        _CACHED["wdata"] = (a1, a2, bvec)
        _CACHED.pop("wdev", None)

    if "nc" not in _CACHED:
        _CACHED["nc"] = build_nc(L)
    nc = _CACHED["nc"]
    st = _runner_state(nc, NCORES)
    devices = st["devices"]

    # device-resident replicated weights (cached across calls; re-put only
    # when the params change)
    if "wdev" not in _CACHED:
        a1, a2, bvec = _CACHED["wdata"]
        wdev = {}
        for name, arr in (("w1l", a1), ("w2l", a2), ("bvec", bvec)):
            futs = [_POOL.submit(jax.device_put, arr, devices[c])
                    for c in range(NCORES)]
            wdev[name] = [f.result() for f in futs]
        _CACHED["wdev"] = wdev
    wdev = _CACHED["wdev"]

    # reusable per-core scratch: int8 padded upload buffers + f32 temp
    if "qbufs" not in _CACHED:
        qb = [np.zeros((2, CIN, R), np.int8) for _ in range(NCORES)]
        _CACHED["qbufs"] = qb
        _CACHED["tbuf"] = [np.empty((2, CIN, L), np.float32) for _ in range(NCORES)]
    qbufs, tbufs = _CACHED["qbufs"], _CACHED["tbuf"]

    # quantize x -> int8 (exactly reference fake_quant * 128), pad 2 cols
    # each side (buffers are zero-initialized once; edges never written).
    # Passed as a lazy callable so the runner quantizes each core right
    # before its upload slot instead of serializing all 8 upfront.
    def quant_core(c):
        def doit():
            import os
            if os.environ.get("KFAKEQ") and _CACHED.get("qdone"):
                return qbufs[c]
            xs = x[2 * c:2 * c + 2]                 # [2,4,L]
            t = tbufs[c]
            np.multiply(xs, np.float32(128.0), out=t)
            t += np.float32(0.5)
            np.floor(t, out=t)
            np.clip(t, -128.0, 127.0, out=t)
            qbufs[c][:, :, 2:2 + L] = t
            return qbufs[c]
        return doit

    out = np.empty((B, 2, L), np.float32)

    def core_post(c, outs_np):
        # dequantize in the per-core fetch thread so it overlaps later
        # cores' downloads
        import os
        if os.environ.get("KFAKEDQ"):
            return [None]
        np.multiply(outs_np[0], INV128, out=out[2 * c:2 * c + 2],
                    dtype=np.float32)
        return [None]

    _CACHED["core_post"] = core_post

    in_maps = []
    for c in range(NCORES):
        in_maps.append({
            "xp": quant_core(c),
            "w1l": wdev["w1l"][c], "w2l": wdev["w2l"][c], "bvec": wdev["bvec"][c],
        })
    res = bass_utils.run_bass_kernel_spmd(nc, in_maps, core_ids=list(range(NCORES)))
    assert all(r["z"] is None for r in res.results)
    _CACHED["qdone"] = True

    wq, bq, w2q, b2q = _CACHED["folded"]
    _edge_fix(out, x, wq, bq, w2q, b2q)
    return out
